# revision 31
# baseline (speedup 1.0000x reference)
"""Bi-directional minGRU Trainium2 kernel.

Full-input contract: kernel(**inputs) takes the unsharded numpy inputs from
reference.setup_inputs() and returns the full (B, L, 1) float32 output.

Sharding: data-parallel over batch B=32 across 8 NeuronCores (4 sequences per
core), parameters replicated. Per core, each sequence is processed in a
feature-on-partition / time-on-free layout:

  t_enc  : relu(t*w1+b1) after a DMA broadcast of t across 64 partitions
           (DVE for sequence 0, ACT afterwards)
  rr     : [t_enc(64) ; x(2)] -> 66 partitions.  All bias-like terms
           (proj bias, te_b2 second-layer bias) are folded into the gate /
           head biases on the host, so no ones row and no memset is needed.
  inp    : composed into the gate weights host-side (fp64)
  z,hb   : rr @ wz / wh on PE into [128, 1024] PSUM tiles,
           sigmoid/tanh+bias evacuation on ACT
  a=1-z  : produced directly by ACT via sigmoid(-(pre+bz))
  scan   : runs over w = h~_cur - state, whose recurrence
           w = (e + w_prev) * a  (e = shifted difference of h~) fits
           tensor_tensor_scan's (d0 add s) mult d1 form — the product
           (1-a)*h~ is never materialised; e is one 2x-mode DVE subtract.
           The backward direction runs the same scan through
           negative-stride APs.
  head   : h = h~_shift - w_shift is reconstructed INSIDE the head GEMM:
           each unit contributes paired +g1/-g1 k-tile passes (9 passes
           total incl. the t_enc tile) into a resident PSUM accumulator,
           emitted per-unit as scans complete (PASS_LAG units behind the
           gates) so only the final pass trails the last scan; relu+bias
           on ACT, then @ gh_w2 on PE; gh_b2 is added on host.
  The PE is kept on-clock by a scratch warm-up burst while the first
  input DMAs land.
"""

import time

import numpy as np
import ml_dtypes

import concourse.bass as bass
import concourse.mybir as mybir
import concourse.tile as tile
from concourse.vector_clock import ScopedClock, VectorClock
from concourse.bass_utils import run_bass_kernel_spmd

# ---------------------------------------------------------------------------
# Workaround for a walrus codegen limit in this toolchain: the TileContext
# tail drain carries one sync-wait per live proc sem, but this walrus build
# rejects >2 sync waits on a Drain (CTRL_NO_STRUCT template). Re-emit the tail
# with the waits split across single-wait NOPs on the sync engine (same-engine
# program order preserves the semantics), followed by a wait-free drain.
# ---------------------------------------------------------------------------


def _patched_drain_and_barrier(self, tick_clock, wait_clock):
    nc = self.nc
    vals = list(tick_clock.global_clock)
    n = len(vals)
    for i, v in enumerate(vals):
        if v > 0:
            partial = [0] * n
            partial[i] = v
            nop = nc.sync.nop()
            wait_clock.add_sem_waits(nop.ins, ScopedClock({None: VectorClock(partial)}))
    nc.sync.drain()
    nc.all_engine_barrier()
    assert self.sems is not None
    popped = nc._tile_sem_poison_stack.pop()
    assert popped is self._sem_poison
    nc.clear_and_free_semaphores(list(self.sems.allocated().values()))
    nc.all_engine_barrier()


tile.TileContext._drain_and_barrier = _patched_drain_and_barrier


def _spill_excess_waits(nc, maxw=1):
    """Split instructions carrying more than `maxw` sem waits: the excess
    waits move onto NoOps inserted just before, on the same engine (same-
    engine program order keeps the semantics identical)."""
    for bb in nc.m.functions[0].blocks:
        new = []
        for inst in bb.instructions:
            si = inst.sync_info
            if si is not None and si.on_wait is not None and len(si.on_wait) > maxw:
                waits = list(si.on_wait)
                excess, keep = waits[:-maxw], waits[-maxw:]
                for j, w in enumerate(excess):
                    nop = mybir.InstNoOp(
                        name=f"{inst.name}_ws{j}",
                        engine=inst.engine,
                        ins=[],
                        outs=[],
                        sync_info=mybir.SyncInfo(on_wait=[w], on_update=[]),
                    )
                    nc.register_instruction(nop)
                    new.append(nop)
                si.on_wait = keep
            new.append(inst)
        if len(new) != len(bb.instructions):
            _replace_block_instructions(bb, new)


def _replace_block_instructions(bb, new):
    try:
        bb.instructions = new
    except Exception:
        while len(bb.instructions):
            bb.instructions.pop()
        for inst in new:
            bb.add_instruction(inst)

# ---------------------------------------------------------------------------

B, L, H, TE = 32, 2048, 256, 64
NCORES = 8
BS = B // NCORES           # sequences per core
HH = 128                   # gauss head hidden
IN_AUG = TE + 2            # rr rows: t_enc(64) + x(2); biases folded on host
F32 = mybir.dt.float32

DT = mybir.dt.bfloat16     # matmul/activation storage dtype
NP_DT = ml_dtypes.bfloat16

FCH = 512                  # matmul moving-operand chunk (one PSUM bank fp32)


def _rev(t, cols, ncols):
    """Reversed-free-dim view of tile AP t over columns [cols, cols+ncols)."""
    return bass.AP(
        tensor=t.tensor,
        offset=t.offset + cols + ncols - 1,
        ap=[list(t.ap[0]), [-1, ncols]],
    )


def _build_nc(bs=BS, repeats=1, psum_cols=1024, psum_bufs=2, mp_bufs=3, hp_bufs=3,
              stt_mode="dve_stt", te1_mode="act", r_engine="act", orow_engine="act"):
    nc = bass.Bass("TRN2", target_bir_lowering=False, debug=False, num_devices=NCORES)

    d_xT = nc.dram_tensor("xT", [bs, 2, L], DT, kind="ExternalInput")
    d_t = nc.dram_tensor("t", [bs, L], DT, kind="ExternalInput")
    # gate weights with the input projection and time-encoder composed in
    # (host-side): operate directly on R = [te1_hidden(64); x(2)]
    d_wz = {d: nc.dram_tensor(f"wz{d}", [IN_AUG, H], DT, kind="ExternalInput")
            for d in "fb"}
    d_wh = {d: nc.dram_tensor(f"wh{d}", [IN_AUG, H], DT, kind="ExternalInput")
            for d in "fb"}
    d_bz = {d: nc.dram_tensor(f"bz{d}", [H, 1], F32, kind="ExternalInput")
            for d in "fb"}
    d_bh = {d: nc.dram_tensor(f"bh{d}", [H, 1], F32, kind="ExternalInput")
            for d in "fb"}
    d_g1 = nc.dram_tensor("g1", [2 * H, HH], DT, kind="ExternalInput")
    d_g1n = nc.dram_tensor("g1n", [2 * H, HH], DT, kind="ExternalInput")
    d_g1te = nc.dram_tensor("g1te", [IN_AUG, HH], DT, kind="ExternalInput")
    d_g2 = nc.dram_tensor("g2", [HH, 1], DT, kind="ExternalInput")
    d_tw1 = nc.dram_tensor("tw1", [TE, 1], F32, kind="ExternalInput")
    d_tb1 = nc.dram_tensor("tb1", [TE, 1], F32, kind="ExternalInput")
    d_gb1 = nc.dram_tensor("gb1", [HH, 1], F32, kind="ExternalInput")
    d_out = nc.dram_tensor("out", [bs, L], F32, kind="ExternalOutput")

    with tile.TileContext(nc) as tc:
        with (
            tc.tile_pool(name="wpool", bufs=1) as wp,
            tc.tile_pool(name="mpool", bufs=mp_bufs) as mp,
            tc.tile_pool(name="hpool", bufs=hp_bufs) as hp,
            tc.tile_pool(name="psum", bufs=psum_bufs, space="PSUM") as pp,
        ):
            ENG = {"v": nc.vector, "g": nc.gpsimd}

            # ---- replicated weights, loaded once ----
            # Queue assignment keeps the critical path short: sync carries the
            # te1 scalars (+ the input DMAs from stage1), gpsimd the gate
            # weights, scalar the head weights (all needed later).
            def wload(shape, dtype, tag, src_ap, eng):
                t = wp.tile(shape, dtype, tag=tag, name=tag)
                eng.dma_start(out=t, in_=src_ap)
                return t

            s_tw1 = wload([TE, 1], F32, "tw1", d_tw1[:, :], nc.sync)
            s_tb1 = wload([TE, 1], F32, "tb1", d_tb1[:, :], nc.sync)
            s_wz, s_wh, s_bz, s_bh = {}, {}, {}, {}
            for d in "fb":
                s_wz[d] = wload([IN_AUG, H], DT, f"wz{d}", d_wz[d][:, :], nc.gpsimd)
                s_wh[d] = wload([IN_AUG, H], DT, f"wh{d}", d_wh[d][:, :], nc.gpsimd)
                s_bz[d] = [wload([128, 1], F32, f"bz{d}{k}",
                                 d_bz[d][128 * k:128 * (k + 1), :], nc.gpsimd)
                           for k in range(2)]
                s_bh[d] = [wload([128, 1], F32, f"bh{d}{k}",
                                 d_bh[d][128 * k:128 * (k + 1), :], nc.gpsimd)
                           for k in range(2)]
            s_g1 = [wload([128, HH], DT, f"g1_{j}", d_g1[128 * j:128 * (j + 1), :],
                          nc.scalar) for j in range(4)]
            s_g1n = [wload([128, HH], DT, f"g1n_{j}", d_g1n[128 * j:128 * (j + 1), :],
                           nc.scalar) for j in range(4)]
            s_g1te = wload([IN_AUG, HH], DT, "g1te", d_g1te[:, :], nc.scalar)
            s_g2 = wload([HH, 1], DT, "g2", d_g2[:, :], nc.scalar)
            s_gb1 = wload([HH, 1], F32, "gb1", d_gb1[:, :], nc.scalar)

            segs = L // psum_cols
            spc = psum_cols // FCH

            def gemm(rows, ktiles, consume):
                """Emit a [rows, L] GEMM in psum_cols segments; ktiles is a
                list of (lhsT, rhs_tile) accumulated along k; consume(ps, c0)
                evacuates each PSUM segment starting at column c0."""
                for seg in range(segs):
                    ps = pp.tile([128, psum_cols], F32, tag="ps", name="ps")
                    for ki, (w, r) in enumerate(ktiles):
                        for ch in range(spc):
                            c0 = seg * psum_cols + ch * FCH
                            nc.tensor.matmul(
                                ps[0:rows, ch * FCH:(ch + 1) * FCH], lhsT=w,
                                rhs=r[:, c0:c0 + FCH],
                                start=(ki == 0), stop=(ki == len(ktiles) - 1))
                    consume(ps, seg * psum_cols)

            # PE p-state warm-up: a burst of scratch matmuls keeps the PE
            # streaming while the input DMAs land, so the first real gemms
            # run at full clock.  Depends only on a memset scratch tile.
            scr = wp.tile([128, FCH], DT, tag="scr", name="scr")
            nc.vector.memset(scr, 0.0)
            ps_w = pp.tile([128, psum_cols], F32, tag="ps", name="ps")
            for i in range(24):
                nc.tensor.matmul(ps_w[:, 0:FCH], lhsT=scr[:, 0:128], rhs=scr,
                                 start=True, stop=True)

            def stage1(bi, mode=None):
                """R = [relu(t*w1+b1)(64) ; x(2)] for sequence bi."""
                mode = mode or te1_mode
                rr = mp.tile([IN_AUG, L], DT, tag="rr", name="rr", bufs=max(2, bs))
                nc.sync.dma_start(out=rr[TE:TE + 2, :], in_=d_xT[bi])
                t_bc = mp.tile([TE, L], DT, tag="t_bc", name="t_bc")
                trow_ap = d_t[bi:bi + 1, :]
                for q in range(4):
                    nc.sync.dma_start(
                        out=t_bc[16 * q:16 * (q + 1), :],
                        in_=bass.AP(tensor=trow_ap.tensor,
                                    offset=trow_ap.offset,
                                    ap=[[0, 16], list(trow_ap.ap[-1])]))
                if mode == "act":
                    nc.scalar.activation(out=rr[0:TE, :], in_=t_bc,
                                         func=mybir.ActivationFunctionType.Relu,
                                         scale=s_tw1, bias=s_tb1)
                else:
                    eng = {"pool": nc.gpsimd, "dve": nc.vector}[mode]
                    eng.tensor_scalar(out=rr[0:TE, :], in0=t_bc,
                                      scalar1=s_tw1, scalar2=s_tb1,
                                      op0=mybir.AluOpType.mult,
                                      op1=mybir.AluOpType.add)
                    eng.tensor_relu(rr[0:TE, :], rr[0:TE, :])
                return rr

            UNITS = [(d, ph) for d in "fb" for ph in range(2)]
            PASS_LAG = 3

            for r in range(repeats):
                # stage 1 for the first sequence only (on DVE: no ACT table
                # load on the critical path); later sequences are prepared
                # just-in-time inside the main loop
                rrs = [stage1(0, mode="dve")]

                # The gauss head accumulates its 5 k-tile passes into a
                # dedicated full-width PSUM region (tag psh) one pass at a
                # time, as each unit's scan completes — only the final pass
                # trails the last scan instead of the whole head.
                psh_of, rt_of, hv_at = {}, {}, {}
                todo = {}

                def sched(slot, fn):
                    todo.setdefault(slot, []).append(fn)

                def drain(slot):
                    for fn in todo.pop(slot, []):
                        fn()

                def head_pass(bi, j, w, rhs, start=False, stop=False):
                    psh = psh_of[bi]
                    for ch in range(L // FCH):
                        nc.tensor.matmul(
                            psh[:, ch * FCH:(ch + 1) * FCH], lhsT=w,
                            rhs=rhs[:, ch * FCH:(ch + 1) * FCH],
                            start=start, stop=stop)

                def make_hv_pass(bi, u):
                    def fn():
                        passes = hv_at[(bi, u)]
                        for pi, (w, rhs) in enumerate(passes):
                            head_pass(bi, u, w, rhs,
                                      stop=(u == 3 and pi == len(passes) - 1))
                        if u == 3:
                            rt = mp.tile([HH, L], DT, tag="rt", name="rt")
                            rt_of[bi] = rt
                            if r_engine == "act":
                                nc.scalar.activation(
                                    out=rt, in_=psh_of[bi][0:HH],
                                    func=mybir.ActivationFunctionType.Relu,
                                    bias=s_gb1)
                            else:
                                ENG[r_engine].tensor_scalar(
                                    out=rt, in0=psh_of[bi][0:HH],
                                    scalar1=s_gb1, scalar2=0.0,
                                    op0=mybir.AluOpType.add,
                                    op1=mybir.AluOpType.max)
                    return fn

                def make_g2(bi):
                    def fn():
                        orow = mp.tile([1, L], F32, tag="orow", name="orow")

                        def o_consume(ps, c0, orow=orow):
                            if orow_engine == "act":
                                nc.scalar.activation(
                                    out=orow[:, c0:c0 + psum_cols], in_=ps[0:1],
                                    func=mybir.ActivationFunctionType.Copy)
                            else:
                                ENG[orow_engine].tensor_copy(
                                    out=orow[:, c0:c0 + psum_cols], in_=ps[0:1])
                        gemm(1, [(s_g2, rt_of[bi])], o_consume)
                        nc.sync.dma_start(out=d_out[bi:bi + 1, :], in_=orow)
                    return fn

                g = 0
                for bi in range(bs):
                    rr = rrs[bi]
                    for u, (d, ph) in enumerate(UNITS):
                        zt = mp.tile([128, L], DT, tag="zt", name="zt")

                        # a = 1-z = sigmoid(-(pre+bz)) straight from ACT
                        # (bz arrives pre-negated from the host)
                        def z_consume(ps, c0, zt=zt, d=d, ph=ph):
                            nc.scalar.activation(
                                out=zt[:, c0:c0 + psum_cols], in_=ps,
                                func=mybir.ActivationFunctionType.Sigmoid,
                                scale=-1.0, bias=s_bz[d][ph])
                        gemm(128, [(s_wz[d][:, 128 * ph:128 * (ph + 1)], rr)],
                             z_consume)
                        at = zt
                        # h~ lands in a (L+1)-wide tile with a zero edge
                        # column so both the shifted-difference input of the
                        # scan and the shifted h~ head k-tile are plain views
                        ht = mp.tile([128, L + 1], DT, tag="ht", name="ht")
                        off = 1 if d == "f" else 0

                        def h_consume(ps, c0, ht=ht, d=d, ph=ph, off=off):
                            nc.scalar.activation(
                                out=ht[:, c0 + off:c0 + off + psum_cols], in_=ps,
                                func=mybir.ActivationFunctionType.Tanh,
                                bias=s_bh[d][ph])
                        gemm(128, [(s_wh[d][:, 128 * ph:128 * (ph + 1)], rr)],
                             h_consume)

                        if u == 1:
                            # prepare the NEXT sequence early so its te1 ACT
                            # op never heads the ACT queue while its input
                            # DMA is still in flight
                            if bi + 1 < bs:
                                rrs.append(stage1(bi + 1))
                        if u == 2:
                            # open this sequence's head accumulator with the
                            # k-tile that only needs rr.  Opened at unit 2 so
                            # the previous sequence's closing pass + relu are
                            # already emitted (psh has a single PSUM buffer).
                            psh_of[bi] = pp.tile([128, L], F32, tag="psh",
                                                 name="psh", bufs=1)
                            head_pass(bi, -1, s_g1te, rr, start=True)

                        # Scan over w = h~_cur - state:  w = (e + w_prev) * a
                        # with e the shifted difference of h~ — the product
                        # (1-a)*h~ never needs materialising.  The hidden
                        # state is reconstructed inside the head GEMM as
                        # h = h~_shift - w_shift via paired +/-g1 k-tiles.
                        et = mp.tile([128, L], DT, tag="et", name="et")
                        if d == "f":
                            nc.gpsimd.memset(ht[:, 0:1], 0.0)
                            nc.vector.tensor_tensor(
                                out=et, in0=ht[:, 1:L + 1], in1=ht[:, 0:L],
                                op=mybir.AluOpType.subtract)
                            wv = hp.tile([128, L], DT, tag=f"w{d}{ph}",
                                         name=f"w{d}{ph}")
                            nc.gpsimd.memset(wv[:, 0:1], 0.0)
                            nc.vector.tensor_tensor_scan(
                                out=wv[:, 1:L], data0=et[:, 0:L - 1],
                                data1=at[:, 0:L - 1], initial=0.0,
                                op0=mybir.AluOpType.add,
                                op1=mybir.AluOpType.mult)
                            hv_at[(bi, u)] = [(s_g1[u], ht[:, 0:L]),
                                              (s_g1n[u], wv[:, 0:L])]
                        else:
                            nc.gpsimd.memset(ht[:, L:L + 1], 0.0)
                            nc.vector.tensor_tensor(
                                out=et, in0=ht[:, 0:L], in1=ht[:, 1:L + 1],
                                op=mybir.AluOpType.subtract)
                            wv = hp.tile([128, L + 1], DT, tag=f"w{d}{ph}",
                                         name=f"w{d}{ph}")
                            nc.gpsimd.memset(wv[:, L:L + 1], 0.0)
                            nc.vector.tensor_tensor_scan(
                                out=_rev(wv, 1, L - 1), data0=_rev(et, 1, L - 1),
                                data1=_rev(at, 1, L - 1), initial=0.0,
                                op0=mybir.AluOpType.add,
                                op1=mybir.AluOpType.mult)
                            hv_at[(bi, u)] = [(s_g1[u], ht[:, 1:L + 1]),
                                              (s_g1n[u], wv[:, 1:L + 1])]

                        sched(g + PASS_LAG, make_hv_pass(bi, u))
                        if u == 3:
                            sched(g + PASS_LAG + 2, make_g2(bi))
                        drain(g)
                        g += 1
                for slot in range(g, g + PASS_LAG + 8):
                    drain(slot)
                assert not todo

    _spill_excess_waits(nc)
    return nc


def _host_prep(inputs):
    """Per-core input maps. The input projection and time-encoder second layer
    are composed into the gate/head weights (fp64) so the device operates
    directly on R = [te1_hidden(64); x(2)]; all bias-like contributions are
    folded into the per-partition gate/head biases."""
    f = {k: np.asarray(v, np.float64) for k, v in inputs.items()}

    def dt(a):
        return np.ascontiguousarray(a.astype(np.float32).astype(NP_DT))

    def f32c(a):
        return np.ascontiguousarray(a.astype(np.float32))

    def gate_w(pw, w):
        """(66,256) weight in the R basis for pre = (xc@[pw;pb]) @ w."""
        te_part = f["te_w2"] @ pw[2:66] @ w              # (64,256)
        x_part = pw[0:2] @ w                             # (2,256)
        return np.concatenate([te_part, x_part], axis=0)

    common = {}
    for d, pw, pb in (("f", f["fproj_w"], f["fproj_b"]),
                      ("b", f["bproj_w"], f["bproj_b"])):
        bias_c = f["te_b2"] @ pw[2:66] + pb              # (256,) inp-basis bias
        common[f"wz{d}"] = dt(gate_w(pw, f[f"{d}wz_w"]))
        common[f"wh{d}"] = dt(gate_w(pw, f[f"{d}wh_w"]))
        common[f"bz{d}"] = f32c(-(bias_c @ f[f"{d}wz_w"] + f[f"{d}wz_b"])[:, None])
        common[f"bh{d}"] = f32c((bias_c @ f[f"{d}wh_w"] + f[f"{d}wh_b"])[:, None])
    common["g1"] = dt(f["gh_w1"][0:2 * H])
    common["g1n"] = dt(-f["gh_w1"][0:2 * H])
    g1te = f["gh_w1"][2 * H:2 * H + TE]                  # (64,128)
    common["g1te"] = dt(np.concatenate(
        [f["te_w2"] @ g1te, np.zeros((2, HH))], axis=0))
    common["g2"] = dt(f["gh_w2"])
    common["tw1"] = f32c(f["te_w1"].T)
    common["tb1"] = f32c(f["te_b1"][:, None])
    common["gb1"] = f32c((f["gh_b1"] + f["te_b2"] @ g1te)[:, None])
    in_maps = []
    for c in range(NCORES):
        sl = slice(BS * c, BS * (c + 1))
        m = dict(common)
        m["xT"] = dt(f["x"][sl].transpose(0, 2, 1))
        m["t"] = dt(f["t"][sl, :, 0])
        in_maps.append(m)
    return in_maps, float(f["gh_b2"][0])


_CACHE = {}


def _get_nc():
    if "nc" not in _CACHE:
        _CACHE["nc"] = _build_nc()
    return _CACHE["nc"]


def kernel(**inputs):
    nc = _get_nc()
    in_maps, gh_b2 = _host_prep(inputs)
    res = run_bass_kernel_spmd(nc, in_maps, list(range(NCORES)))
    out = np.empty((B, L, 1), np.float32)
    for c in range(NCORES):
        out[BS * c:BS * (c + 1), :, 0] = res.results[c]["out"] + gh_b2
    return out


def _build_sharded_exec(nc):
    """Non-donating clone of bass2jax.run_bass_via_pjrt's multi-core path so
    the executable can be launched repeatedly for timing."""
    import jax
    import concourse.mybir as mb
    from jax.experimental.shard_map import shard_map
    from jax.sharding import Mesh, PartitionSpec
    from concourse import bass2jax

    bass2jax.install_neuronx_cc_hook()
    part_name = nc.partition_id_tensor.name if nc.partition_id_tensor else None
    in_names, out_names, out_avals, zero_outs = [], [], [], []
    for alloc in nc.m.functions[0].allocations:
        if not isinstance(alloc, mb.MemoryLocationSet):
            continue
        name = alloc.memorylocations[0].name
        if alloc.kind == "ExternalInput":
            if name != part_name:
                in_names.append(name)
        elif alloc.kind == "ExternalOutput":
            shape = tuple(alloc.tensor_shape)
            dtype = mb.dt.np(alloc.dtype)
            out_names.append(name)
            out_avals.append(jax.core.ShapedArray(shape, dtype))
            zero_outs.append(np.zeros(shape, dtype))
    n_params = len(in_names)
    all_names = in_names + out_names
    if part_name is not None:
        all_names = all_names + [part_name]

    def _body(*args):
        operands = list(args)
        if part_name is not None:
            operands.append(bass2jax.partition_id_tensor())
        outs = bass2jax._bass_exec_p.bind(
            *operands,
            out_avals=tuple(out_avals),
            in_names=tuple(all_names),
            out_names=tuple(out_names),
            lowering_input_output_aliases=(),
            sim_require_finite=True,
            sim_require_nnan=True,
            nc=nc,
        )
        return tuple(outs)

    devices = jax.devices()[:NCORES]
    mesh = Mesh(np.asarray(devices), ("core",))
    nin = n_params + len(out_names)
    sharded = jax.jit(
        shard_map(_body, mesh=mesh,
                  in_specs=(PartitionSpec("core"),) * nin,
                  out_specs=(PartitionSpec("core"),) * len(out_names),
                  check_rep=False),
        keep_unused=True,
    )
    return sharded, mesh, in_names, out_names, zero_outs


def _timed_launch(nc, in_maps, iters):
    import jax
    from jax.sharding import NamedSharding, PartitionSpec

    sharded, mesh, in_names, out_names, zero_outs = _build_sharded_exec(nc)
    sh = NamedSharding(mesh, PartitionSpec("core"))
    concat_in = [
        np.concatenate([np.asarray(in_maps[c][n]) for c in range(NCORES)], axis=0)
        for n in in_names
    ]
    concat_zero = [
        np.zeros((NCORES * z.shape[0], *z.shape[1:]), z.dtype) for z in zero_outs
    ]
    args = [jax.device_put(a, sh) for a in concat_in + concat_zero]
    out = sharded(*args)
    jax.block_until_ready(out)
    ts = []
    for _ in range(iters):
        t0 = time.perf_counter()
        out = sharded(*args)
        jax.block_until_ready(out)
        ts.append(time.perf_counter() - t0)
    return min(ts)


def bench(inputs, iters=10, r_hi=5):
    """Estimate on-device kernel time (ns) free of launch overhead: build the
    same kernel with the per-core work repeated 1x and r_hi x inside one NEFF
    and report the slope ((t_hi - t_1) / (r_hi - 1))."""
    in_maps, _ = _host_prep(inputs)
    t1 = _timed_launch(_build_nc(repeats=1), in_maps, iters)
    th = _timed_launch(_build_nc(repeats=r_hi), in_maps, iters)
    print(f"bench: launch r=1 {t1*1e6:.0f} us, r={r_hi} {th*1e6:.0f} us")
    return (th - t1) / (r_hi - 1) * 1e9
